# revision 10
# baseline (speedup 1.0000x reference)
"""Trainium2 Bass kernel for nn_BinaryTreeLogicNet (final).

y = sigmoid(fc_w * root(x) + fc_b), where root() is a 10-level binary-tree
reduction over the 1024 features; each merge (params w0, w1, bias;
lam = sigmoid(bias), om = 1 - lam):
    a = |w0 * node_e| + EPS,  b = |w1 * node_o| + EPS
    out = a^lam * b^(1-lam) + om * max(a, b)
        = exp(lam*ln a + om*ln b) + max(om|w0|node_e, om|w1|node_o) + om*EPS

Sharding: pure data parallel — batch split across 8 NeuronCores (8192
rows/core); merge params replicated, preprocessed on host into lhsT
matrices / per-partition scale vectors (bf16/f32).

Device design (node-on-partition layout, batch on the free dim, bf16):
  phase T   x batch tiles DMA'd with f32->bf16 cast (SWDGE), transposed
            128x128 on TensorE into PSUM, evacuated with |.| via a
            bitwise-and tensor_scalar into the node-major working set.
  per level (tau = one 128-slot output tile; sigma = input tile):
    ACT     lab = Ln(|w|_pp * cur + EPS)       per-partition scale/bias
    PE      T = lam*la + om*lb ; A = om|w0|cur_e ; B = om|w1|cur_o
            (host-built bf16 lhsT pair-combining matmuls, fp32 PSUM)
    ACT     p = Exp(T)
    DVE     sb = B + omeps_pp                  (tensor_scalar)
    DVE     u  = (A + omeps_pp) max sb         (scalar_tensor_tensor)
    DVE/GpS cur' = u + p                       (tensor_add)
  Deep levels (width < 128/chunk) pack all chunks' nodes onto partitions;
  every tau is emitted at the chunk where its inputs become ready, so the
  tree tail overlaps the batch pipeline. Ln/Exp are pinned to the
  natural_log_exp_and_others ACT table set (avoids per-op table reloads).
  Head: ACT Sigmoid directly on the root tile.
"""

import sys

if "/opt/trn_rl_repo" not in sys.path:
    sys.path.insert(0, "/opt/trn_rl_repo")

import numpy as np

import ml_dtypes

import concourse.bacc as bacc
import concourse.hw_specs as hw_specs
import concourse.mybir as mybir
import concourse.tile as tile
from concourse.bass_utils import run_bass_kernel_spmd

F32 = mybir.dt.float32
U32 = mybir.dt.uint32
BF16 = mybir.dt.bfloat16
U16 = mybir.dt.uint16
ALU = mybir.AluOpType
ACTF = mybir.ActivationFunctionType

N = 1024
B = 65536
NCORES = 8
ROWS_PER_CORE = B // NCORES  # 8192
EPS = 1e-6

# defaults for the full-size kernel
F_CHUNK = 1024  # batch columns per chunk
FPE = 512  # matmul free dim per instruction (one PSUM bank)

_COMBINED_SET = "natural_log_exp_and_others"
_tables_patched = False


def _patch_act_tables():
    """Keep Ln/Exp resolvable only via the combined table set so the
    table-load pass doesn't alternate between per-function sets (each
    reload costs ~2.7us). Dict order (= set ids) is preserved."""
    global _tables_patched
    if _tables_patched:
        return
    orig = hw_specs.get_activation_tables

    def patched(module_arch):
        t = orig(module_arch)
        ln, ex = mybir.ActivationFunctionType.Ln, mybir.ActivationFunctionType.Exp
        for name, fns in t.items():
            if name != _COMBINED_SET:
                fns.discard(ln)
                fns.discard(ex)
        return t

    hw_specs.get_activation_tables = patched
    import concourse.bacc as _b

    if getattr(_b, "get_activation_tables", None) is orig:
        _b.get_activation_tables = patched
    _tables_patched = True


# per-level offsets into the flat (N-1) merge-param arrays
LEVEL_OFF = []
_off = 0
_n = N
while _n > 1:
    LEVEL_OFF.append(_off)
    _off += _n // 2
    _n //= 2
WIDTH = [N >> l for l in range(11)]  # nodes per chunk entering level l (l=10: root)


# ---------------------------------------------------------------------------
# Plan: slot maps + matrix/const layout (shared by builder and host fill)
# ---------------------------------------------------------------------------


class Plan:
    def __init__(self, nch):
        self.nch = nch
        # mats: content-key -> (col, qn) ; fill info kept alongside
        self.mat_cols = {}
        self.mat_fill = []  # (col, qn, kind, entries) for host fill
        self.mat_cursor = 0
        # scb: content-key -> col
        self.scb_cols = {}
        self.scb_fill = []  # (col, entries, kind)
        self.scb_cursor = 0
        self.eye_col = self._alloc_mat(
            "eye", tuple((q, q, -1) for q in range(128)), 128
        )
        self.levels = []  # per level: list of tau group dicts
        self._build()

    def _alloc_mat(self, kind, entries, qn):
        key = (kind, entries, qn)
        if key in self.mat_cols:
            return self.mat_cols[key]
        col = self.mat_cursor
        self.mat_cols[key] = col
        self.mat_fill.append((col, qn, kind, entries))
        self.mat_cursor += qn
        return col

    def _alloc_scb(self, kind, entries):
        key = (kind, entries)
        if key in self.scb_cols:
            return self.scb_cols[key]
        col = self.scb_cursor
        self.scb_cols[key] = col
        self.scb_fill.append((col, entries, kind))
        self.scb_cursor += 1
        return col

    def tile_of(self, l, c, j):
        """Slot of node j (chunk c) in the level-l node set."""
        w = WIDTH[l]
        if l == 0:
            return ("AX", c, j // 128), j % 128
        if w >= 128:
            return ("W", l, c, j // 128), j % 128
        s = c * w + j
        return ("D", l, s // 128), s % 128

    def tiles_of_set(self, l):
        """All (tile_key, kn) for the level-l node set (kn = used partitions)."""
        w = WIDTH[l]
        out = []
        if l == 0:
            for c in range(self.nch):
                for g in range(w // 128):
                    out.append((("AX", c, g), 128))
        elif w >= 128:
            for c in range(self.nch):
                for t in range(w // 128):
                    out.append((("W", l, c, t), 128))
        else:
            tot = w * self.nch
            nt = (tot + 127) // 128
            for t in range(nt):
                out.append((("D", l, t), min(128, tot - 128 * t)))
        return out

    def _build(self):
        for l in range(10):
            m = WIDTH[l + 1]
            groups = {}
            for c in range(self.nch):
                for j in range(m):
                    t0, p0 = self.tile_of(l, c, 2 * j)
                    t1, p1 = self.tile_of(l, c, 2 * j + 1)
                    assert t0 == t1, (l, c, j, t0, t1)
                    tau, q = self.tile_of(l + 1, c, j)
                    groups.setdefault(tau, {}).setdefault(t0, []).append(
                        (q, p0, p1, LEVEL_OFF[l] + j)
                    )
            # ln-scale vectors per sigma tile of set l
            ln_cols = {}
            for sig, kn in self.tiles_of_set(l):
                entries = []
                for c in range(self.nch):
                    for i in range(WIDTH[l]):
                        tk, p = self.tile_of(l, c, i)
                        if tk == sig:
                            entries.append((p, i % 2, LEVEL_OFF[l] + i // 2))
                entries = tuple(sorted(set(entries)))
                ln_cols[sig] = (self._alloc_scb("lnscale", entries), kn)

            taus = []
            for tau in sorted(groups.keys(), key=str):
                sigmas = []
                qmin, qmax = 128, 0
                om_entries = []
                for sig in sorted(groups[tau].keys(), key=str):
                    ms = sorted(groups[tau][sig])
                    qlo = min(q for q, _, _, _ in ms)
                    qhi = max(q for q, _, _, _ in ms) + 1
                    kn = max(max(p0, p1) for _, p0, p1, _ in ms) + 1
                    qmin, qmax = min(qmin, qlo), max(qmax, qhi)
                    ents = tuple((q - qlo, p0, p1, j) for q, p0, p1, j in ms)
                    sigmas.append(
                        dict(
                            sig=sig,
                            qlo=qlo,
                            qhi=qhi,
                            kn=kn,
                            ln_col=ln_cols[sig][0],
                            col_T=self._alloc_mat("T", ents, qhi - qlo),
                            col_A=self._alloc_mat("A", ents, qhi - qlo),
                            col_B=self._alloc_mat("B", ents, qhi - qlo),
                        )
                    )
                    om_entries += [(q, j) for q, _, _, j in ms]
                om_col = self._alloc_scb("omeps", tuple(sorted(om_entries)))
                taus.append(
                    dict(tau=tau, qlo=qmin, qhi=qmax, sigmas=sigmas, om_col=om_col)
                )
            self.levels.append(taus)

    # -- host-side fill ----------------------------------------------------
    def fill(self, weights, biases):
        w64 = weights.astype(np.float64)
        lam = 1.0 / (1.0 + np.exp(-biases.astype(np.float64)))
        om = 1.0 - lam
        w0a, w1a = np.abs(w64[:, 0]), np.abs(w64[:, 1])
        mats = np.zeros((128, self.mat_cursor), dtype=ml_dtypes.bfloat16)
        for col, qn, kind, entries in self.mat_fill:
            if kind == "eye":
                for q, _, _ in entries:
                    mats[q, col + q] = 1.0
                continue
            for q, p0, p1, j in entries:
                if kind == "T":
                    mats[p0, col + q] = lam[j]
                    mats[p1, col + q] = om[j]
                elif kind == "A":
                    mats[p0, col + q] = om[j] * w0a[j]
                elif kind == "B":
                    mats[p1, col + q] = om[j] * w1a[j]
        scb = np.zeros((128, max(self.scb_cursor, 1)), dtype=np.float32)
        for col, entries, kind in self.scb_fill:
            if kind == "lnscale":
                for p, side, j in entries:
                    scb[p, col] = w0a[j] if side == 0 else w1a[j]
            else:  # omeps
                for q, j in entries:
                    scb[q, col] = om[j] * EPS
        return mats, scb


# ---------------------------------------------------------------------------
# Kernel builder
# ---------------------------------------------------------------------------


def build_kernel(rows, fc_w, fc_b, F=F_CHUNK):
    nch = rows // F
    plan = Plan(nch)
    nbt = F // 128  # batch tiles per chunk

    _patch_act_tables()
    nc = bacc.Bacc("TRN2", target_bir_lowering=False)
    x_d = nc.declare_dram_parameter("x", [rows, N], F32, isOutput=False)
    mats_d = nc.declare_dram_parameter(
        "mats", [128, plan.mat_cursor], BF16, isOutput=False
    )
    scb_d = nc.declare_dram_parameter(
        "scb", [128, max(plan.scb_cursor, 1)], F32, isOutput=False
    )
    y_d = nc.declare_dram_parameter("y", [nch, F], F32, isOutput=True)

    # readiness: the chunk index after which each tau's inputs are complete.
    # W/AX tiles are ready at their own chunk; D tiles when every tau writing
    # them has run. Emitting each tau at its readiness chunk spreads the deep
    # levels across the run instead of leaving a serial tail.
    tile_ready = {}
    tau_ready = {}
    for l in range(10):
        for tg in plan.levels[l]:
            r = 0
            for sg in tg["sigmas"]:
                sig = sg["sig"]
                if sig[0] == "AX":
                    r = max(r, sig[1])
                elif sig[0] == "W":
                    r = max(r, sig[2])
                else:
                    r = max(r, tile_ready[sig])
            tau_ready[id(tg)] = r
            tau = tg["tau"]
            if tau[0] == "D":
                tile_ready[tau] = max(tile_ready.get(tau, 0), r)

    def tau_home(tg):
        return tau_ready[id(tg)]

    with tile.TileContext(nc) as tc:
        with (
            tc.tile_pool(name="c1", bufs=1) as pool1,
            tc.tile_pool(name="c2", bufs=2) as pool2,
            tc.tile_pool(name="c3", bufs=3) as pool3,
            tc.tile_pool(name="ps2", bufs=2, space="PSUM") as psum2,
        ):
            mats = pool1.tile([128, plan.mat_cursor], BF16, tag="mats")
            nc.sync.dma_start(out=mats[:], in_=mats_d[:])
            scb = pool1.tile([128, max(plan.scb_cursor, 1)], F32, tag="scb")
            nc.sync.dma_start(out=scb[:], in_=scb_d[:])
            epsb = pool1.tile([128, 1], F32, tag="epsb")
            nc.gpsimd.memset(epsb[:], EPS)
            headb = pool1.tile([128, 1], F32, tag="headb")
            nc.gpsimd.memset(headb[:], float(fc_b))
            eye = mats[:, plan.eye_col : plan.eye_col + 128]

            # deep tiles live across the whole kernel
            tiles = {}
            for l in range(4, 11):
                if WIDTH[l] < 128:
                    for tk, kn in plan.tiles_of_set(l):
                        tiles[tk] = pool1.tile([128, F], BF16, tag=str(tk), name=f"D{tk[1]}_{tk[2]}")

            def sig_ap(tk):
                if tk[0] == "AX":
                    _, c, g = tk
                    return tiles[("AXC", c)][:, g * F : (g + 1) * F]
                return tiles[tk][:]

            def phase_t(c):
                axc = pool2.tile([128, 8 * F], BF16, tag="axT", bufs=2, name=f"axT{c}")
                tiles[("AXC", c)] = axc
                for bt in range(nbt):
                    xb = pool3.tile([128, N], BF16, tag="xb", bufs=4, name=f"xb{c}_{bt}")
                    r0 = c * F + bt * 128
                    nc.gpsimd.dma_start(out=xb[:], in_=x_d[r0 : r0 + 128, :])
                    xps = psum2.tile([128, N], BF16, tag="xps", name=f"xps{c}_{bt}")
                    for g in range(8):
                        nc.tensor.transpose(
                            xps[:, g * 128 : (g + 1) * 128],
                            xb[:, g * 128 : (g + 1) * 128],
                            eye,
                        )
                    dst = (
                        axc[:]
                        .bitcast(U16)
                        .rearrange("p (g f) -> p g f", g=8)[
                            :, :, bt * 128 : (bt + 1) * 128
                        ]
                    )
                    src_ap = xps[:].bitcast(U16).rearrange("p (g f) -> p g f", g=8)
                    nc.vector.tensor_scalar(
                        out=dst,
                        in0=src_ap,
                        scalar1=0x7FFF,
                        scalar2=None,
                        op0=ALU.bitwise_and,
                    )

            def levels_of(c):
                for l in range(1, 4):
                    if WIDTH[l] >= 128:
                        for t in range(WIDTH[l] // 128):
                            tiles[("W", l, c, t)] = pool2.tile(
                                [128, F], BF16, tag=f"W{l}_{t}", name=f"W{l}_{c}_{t}"
                            )
                for l in range(10):
                    for tg in plan.levels[l]:
                        if tau_home(tg) == c:
                            _emit_tau(
                                nc, plan, tg, mats, scb, epsb, pool2, psum2, F, sig_ap
                            )

            phase_t(0)
            for c in range(nch):
                if c + 1 < nch:
                    phase_t(c + 1)
                levels_of(c)

            # head: y = Sigmoid(fc_w*root + fc_b)
            root = tiles[("D", 10, 0)]
            kn = plan.tiles_of_set(10)[0][1]
            yy = pool1.tile([128, F], F32, tag="yy")
            nc.scalar.activation(
                out=yy[0:kn, :],
                in_=root[0:kn, :],
                func=ACTF.Sigmoid,
                bias=headb[0:kn],
                scale=float(fc_w),
            )
            nc.sync.dma_start(out=y_d[:], in_=yy[0:kn, :])
    return nc, plan


_op3_counter = [0]


def _emit_tau(nc, plan, tg, mats, scb, epsb, pool2, psum2, F, sig_ap):
    """One tau group.

    out = max(A + omeps, B + omeps) + p:
      sbB = (B_ps add omeps)            [DVE tensor_scalar, bf16 out]
      u   = (A_ps add omeps) max sbB    [DVE scalar_tensor_tensor, bf16 out]
      cur = u + p                       [tensor_add, bf16; DVE/GpSimd alternating]
    A/B/T matmuls are independent of Exp, keeping the PE stream dense.
    """
    labs = {}
    for sg in tg["sigmas"]:
        kn = sg["kn"]
        lab = pool2.tile([128, F], BF16, tag="lab", bufs=4)
        sc = scb[0:kn, sg["ln_col"] : sg["ln_col"] + 1]
        nc.scalar.activation(
            out=lab[0:kn, :],
            in_=sig_ap(sg["sig"])[0:kn, :],
            func=ACTF.Ln,
            bias=epsb[0:kn],
            scale=sc,
        )
        labs[id(sg)] = lab
    qlo_t, qhi_t = tg["qlo"], tg["qhi"]
    p_t = pool2.tile([128, F], BF16, tag="p", bufs=4)
    sb_t = pool2.tile([128, F], BF16, tag="sb", bufs=4)
    u_t = pool2.tile([128, F], BF16, tag="u", bufs=4)
    omc = tg["om_col"]
    omap = scb[qlo_t:qhi_t, omc : omc + 1]

    # A/B matmuls + DVE combines per PSUM-bank sub-chunk
    for fo in range(0, F, FPE):
        A_ps = psum2.tile([128, FPE], F32, tag="A")
        B_ps = psum2.tile([128, FPE], F32, tag="B")
        for sg in tg["sigmas"]:
            qlo, qhi, kn = sg["qlo"], sg["qhi"], sg["kn"]
            qn = qhi - qlo
            rhs = sig_ap(sg["sig"])[0:kn, fo : fo + FPE]
            nc.tensor.matmul(
                A_ps[qlo:qhi, :],
                mats[0:kn, sg["col_A"] : sg["col_A"] + qn],
                rhs,
                start=True,
                stop=True,
            )
            nc.tensor.matmul(
                B_ps[qlo:qhi, :],
                mats[0:kn, sg["col_B"] : sg["col_B"] + qn],
                rhs,
                start=True,
                stop=True,
            )
        nc.vector.tensor_scalar(
            out=sb_t[qlo_t:qhi_t, fo : fo + FPE],
            in0=B_ps[qlo_t:qhi_t, :],
            scalar1=omap,
            scalar2=None,
            op0=ALU.add,
        )
        nc.vector.scalar_tensor_tensor(
            out=u_t[qlo_t:qhi_t, fo : fo + FPE],
            in0=A_ps[qlo_t:qhi_t, :],
            scalar=omap,
            in1=sb_t[qlo_t:qhi_t, fo : fo + FPE],
            op0=ALU.add,
            op1=ALU.max,
        )

    # T matmuls across the full F into a 2-bank PSUM tile, one Exp at FD=F
    T_ps = psum2.tile([128, F], F32, tag="T", bufs=1)
    for fo in range(0, F, FPE):
        for sg in tg["sigmas"]:
            qlo, qhi, kn = sg["qlo"], sg["qhi"], sg["kn"]
            qn = qhi - qlo
            lab_rhs = labs[id(sg)][0:kn, fo : fo + FPE]
            nc.tensor.matmul(
                T_ps[qlo:qhi, fo : fo + FPE],
                mats[0:kn, sg["col_T"] : sg["col_T"] + qn],
                lab_rhs,
                start=True,
                stop=True,
            )
    nc.scalar.activation(
        out=p_t[qlo_t:qhi_t, :],
        in_=T_ps[qlo_t:qhi_t, :],
        func=ACTF.Exp,
    )

    deep_tau = tg["tau"][0] == "D"
    for k, fo in enumerate(range(0, F, FPE)):
        if deep_tau:
            eng = nc.vector
        else:
            eng = nc.gpsimd if (k + _op3_counter[0]) % 4 != 3 else nc.vector
        eng.tensor_add(
            out=sig_ap(tg["tau"])[qlo_t:qhi_t, fo : fo + FPE],
            in0=u_t[qlo_t:qhi_t, fo : fo + FPE],
            in1=p_t[qlo_t:qhi_t, fo : fo + FPE],
        )
    _op3_counter[0] += 1


def _make_in_maps(x, weights, biases, rows, ncores, F=F_CHUNK):
    plan = Plan(rows // F)
    mats, scb = plan.fill(np.asarray(weights), np.asarray(biases))
    in_maps = []
    for c in range(ncores):
        shard = np.ascontiguousarray(x[c * rows : (c + 1) * rows])
        in_maps.append({"x": shard, "mats": mats, "scb": scb})
    return in_maps


def run_spmd(x, weights, biases, fc_w, fc_b, **spmd_kwargs):
    x = np.asarray(x)
    fc_w = float(np.asarray(fc_w))
    fc_b = float(np.asarray(fc_b))
    nc, plan = build_kernel(ROWS_PER_CORE, fc_w, fc_b)
    in_maps = _make_in_maps(x, weights, biases, ROWS_PER_CORE, NCORES)
    if not nc.is_finalized():
        nc.finalize()
    res = run_bass_kernel_spmd(nc, in_maps, list(range(NCORES)), **spmd_kwargs)
    outs = []
    for c in range(NCORES):
        yc = res.results[c]["y"]  # [nch, F] -> rows c*F+f in order
        outs.append(yc.reshape(-1, 1))
    y = np.ascontiguousarray(np.concatenate(outs, axis=0).astype(np.float32))
    return y, res


def kernel(x, weights, biases, fc_w, fc_b):
    y, _ = run_spmd(x, weights, biases, fc_w, fc_b)
    return y
